# revision 15
# baseline (speedup 1.0000x reference)
"""Trainium2 Bass kernel for a 2-layer LSTM (B=512, T=1024, D=128, H=256, OUT=1).

Strategy: data-parallel over batch (8 cores x 64 rows). Each core runs the full
T=1024 recurrence on its batch shard. On-chip layout is "transposed": partition
dim = feature chunk (128 wide), free dim = 64*chunk_idx + batch, so h-state
tiles are directly the moving (rhs) operand of the recurrent matmuls.

v4 design (per layer, per step):
- ONE PSUM bank [128, 512] holds all 8 gate chunks in order [i i f f o o g g];
  g-gate weight columns are pre-scaled x2 on host so a SINGLE 512-wide sigmoid
  yields sigmoid for i,f,o and s = sigmoid(2*ghat) for g, using the identity
  tanh(x) = 2*sigmoid(2x) - 1.
- Cell state is stored offset+scaled: m = c/2 + 1/2 (in [0,1]), parked in the
  SAME ping-pong tile as the sigmoid output, right after the g region:
  tile layout [i | f | o | s | m] = [128, 640]. Then
    fcig = (in0 - 0.5) * in1   with in0 = [s|m], in1 = [i|f]   (one 256-wide
                               scalar_tensor_tensor: = [i*(s-.5) | f*(m-.5)])
    m_new = (fc + 0.5) + ig    (one 128-wide scalar_tensor_tensor)
    tanh(c) = Tanh(2*m - 1)    (free scale/bias on the ACT instruction)
    h = o * tanh(c)
  This is 3 DVE ops + 2 ACT ops per layer per step (v1: 4-5 DVE + 3 ACT).
- Layer 2 runs LAGGED by one pipeline iteration: its sigmoid/tanh occupy the
  ACT-engine idle slots of the L1 recurrence cycle instead of delaying it.
  Engine-queue orders are pinned with explicit deps (the Tile scheduler
  otherwise reorders ACT/DVE and puts L2 ops on the L1 critical cycle).
- A few always-ready dummy matmuls pad the PE queue right before each rec
  group: they absorb the ~190ns post-idle PE pipeline restart that otherwise
  lands on the critical path, and keep PE activity above the HAM clock-gate
  threshold (otherwise the PE oscillates between 1.2 and 2.4 GHz).
"""

import numpy as np
import ml_dtypes

B, T, D = 512, 1024, 128
H = 256
NCORES = 8
BL = B // NCORES  # 64 batch rows per core
XBLK = 16  # timesteps per x DMA block
# on-chip chunk order [f0 f1 i0 i1 o0 o1 g0 g1]; original order f i g o.
PERM = [0, 1, 2, 3, 6, 7, 4, 5]
G_POS = (6, 7)  # on-chip chunk positions holding the g gate (weights x2)
NDUMMY = 0  # dummies raised chip power -> global P-state downclock; disabled

_F16 = np.float16


def _build(t_steps, with_b1, with_b2):
    import concourse.bass as bass  # noqa: F401
    from concourse.tile import add_dep_helper
    import concourse.mybir as mybir
    import concourse.tile as tile
    from concourse import bacc

    dt = mybir.dt
    AF = mybir.ActivationFunctionType
    ALU = mybir.AluOpType
    nblk = (t_steps + XBLK - 1) // XBLK
    T_ = t_steps

    nc = bacc.Bacc("TRN2", target_bir_lowering=False, debug=False, num_devices=NCORES)
    x_in = nc.declare_dram_parameter(
        "x", [nblk, 128, XBLK, BL], dt.float16, isOutput=False
    )
    w1_in = nc.declare_dram_parameter("w1", [128, 3 * 8 * 128], dt.bfloat16, isOutput=False)
    w2_in = nc.declare_dram_parameter("w2", [128, 4 * 8 * 128], dt.bfloat16, isOutput=False)
    if with_b1:
        b1_in = nc.declare_dram_parameter("b1", [8, 128], dt.bfloat16, isOutput=False)
    if with_b2:
        b2_in = nc.declare_dram_parameter("b2", [8, 128], dt.bfloat16, isOutput=False)
    if with_b1 or with_b2:
        ind_in = nc.declare_dram_parameter("ind", [8, 512], dt.float16, isOutput=False)
    y_out = nc.declare_dram_parameter("y", [128, 128], dt.float32, isOutput=True)

    with tile.TileContext(nc) as tc:
        with (
            tc.tile_pool(name="singles", bufs=1) as singles,
            tc.tile_pool(name="temps", bufs=8) as temps,
            tc.tile_pool(name="psum", bufs=1, space="PSUM") as psum,
        ):
            w1 = singles.tile([128, 3 * 8 * 128], dt.bfloat16)
            w2 = singles.tile([128, 4 * 8 * 128], dt.bfloat16)
            nc.sync.dma_start(out=w1, in_=w1_in[:])
            nc.sync.dma_start(out=w2, in_=w2_in[:])
            if with_b1:
                b1s = singles.tile([8, 128], dt.bfloat16)
                nc.sync.dma_start(out=b1s, in_=b1_in[:])
            if with_b2:
                b2s = singles.tile([8, 128], dt.bfloat16)
                nc.sync.dma_start(out=b2s, in_=b2_in[:])
            if with_b1 or with_b2:
                ind = singles.tile([8, 512], dt.float16)
                nc.sync.dma_start(out=ind, in_=ind_in[:])

            xr = [
                singles.tile([128, XBLK * BL], dt.float16, name=f"xr{i}")
                for i in range(3)
            ]
            h1r = [singles.tile([128, 128], dt.float16, name=f"h1r{i}") for i in range(2)]
            h2r = [singles.tile([128, 128], dt.float16, name=f"h2r{i}") for i in range(2)]
            cg1 = singles.tile([128, 256], dt.float16)  # [c | tanh(g)] co-tile
            cg2 = singles.tile([128, 256], dt.float16)
            out_sb = singles.tile([128, 128], dt.float32)
            for tl in (h1r[0], h1r[1], h2r[0], h2r[1], cg1, cg2):
                nc.gpsimd.memset(tl, 0.0)

            gb1 = [psum.tile([128, 512], dt.float32, name=f"gb1_{i}") for i in range(2)]
            gb2 = [psum.tile([128, 512], dt.float32, name=f"gb2_{i}") for i in range(2)]
            dmy = psum.tile([128, 64], dt.float32, name="dmy")

            nc.sync.dma_start(out=xr[0], in_=x_in[0])

            mm = nc.tensor.matmul

            def w1_tile(k, j):
                i = (k * 8 + j) * 128
                return w1[:, i : i + 128]

            def w2_tile(k, j):
                i = (k * 8 + j) * 128
                return w2[:, i : i + 128]

            def xs_of(t):
                blk = t // XBLK
                tt = t % XBLK
                return xr[blk % 3][:, tt * BL : (tt + 1) * BL]

            def dummies():
                for _ in range(NDUMMY):
                    mm(dmy[:16, :], w1[:, 0:16], w1[:, 0:64],
                       start=True, stop=True, skip_group_check=True)

            def ew(cg, gb, t, h_out, name):
                """fused elementwise: one sigmoid + fixup/mul/add + tanh + h"""
                figog = temps.tile([128, 512], dt.float16, name="figog" + name)
                sig_inst = nc.scalar.activation(figog, gb, AF.Sigmoid)
                # g = 2*sigmoid(2*ghat) - 1 == tanh(ghat)
                nc.vector.tensor_scalar(
                    cg[:, 128:256], figog[:, 384:512], 2.0, 1.0, ALU.mult,
                    ALU.subtract)
                fcig = temps.tile([128, 256], dt.float16, name="fcig" + name)
                # (f|i) * (c|g) = (fc | ig)
                nc.vector.tensor_mul(fcig, figog[:, 0:256], cg)
                nc.vector.tensor_add(cg[:, 0:128], fcig[:, 0:128], fcig[:, 128:256])
                th = temps.tile([128, 128], dt.float16, name="th" + name)
                tc_inst = nc.scalar.activation(th, cg[:, 0:128], AF.Tanh)
                nc.vector.tensor_mul(h_out[:, 0:64], figog[:, 256:320], th[:, 0:64])
                nc.vector.tensor_mul(h_out[:, 64:128], figog[:, 320:384], th[:, 64:128])
                return tc_inst, sig_inst, figog, th

            def emit_l1(t):
                p = t % 2
                blk = t // XBLK
                tt = t % XBLK
                if tt == 0 and blk + 1 < nblk:
                    nc.sync.dma_start(out=xr[(blk + 1) % 3], in_=x_in[blk + 1])
                xs = xs_of(t)
                dummies()
                for j in range(8):
                    mm(gb1[p][:, 64 * j : 64 * j + 64], w1_tile(0, j), xs,
                       start=(j == 0), stop=False, skip_group_check=True)
                if with_b1:
                    mm(gb1[p][:, :], b1s, ind, start=False, stop=False,
                       skip_group_check=True)
                h1_prev = h1r[(t + 1) % 2]
                for k in (1, 2):
                    hk = h1_prev[:, 64 * (k - 1) : 64 * k]
                    for j in range(8):
                        mm(gb1[p][:, 64 * j : 64 * j + 64], w1_tile(k, j), hk,
                           start=False, stop=(k == 2 and j == 7),
                           skip_group_check=True)
                tc_inst, _, _, _ = ew(cg1, gb1[p][:, :], t, h1r[t % 2], "1")
                return tc_inst

            def emit_l2(t, tc1_inst=None):
                p = t % 2
                h1_cur = h1r[t % 2]
                h2_prev = h2r[(t + 1) % 2]
                for k in (0, 1):
                    hk = h1_cur[:, 64 * k : 64 * (k + 1)]
                    for j in range(8):
                        mm(gb2[p][:, 64 * j : 64 * j + 64], w2_tile(k, j), hk,
                           start=(k == 0 and j == 0), stop=False,
                           skip_group_check=True)
                if with_b2:
                    mm(gb2[p][:, :], b2s, ind, start=False, stop=False,
                       skip_group_check=True)
                for k in (2, 3):
                    hk = h2_prev[:, 64 * (k - 2) : 64 * (k - 1)]
                    for j in range(8):
                        mm(gb2[p][:, 64 * j : 64 * j + 64], w2_tile(k, j), hk,
                           start=False, stop=(k == 3 and j == 7),
                           skip_group_check=True)
                _, s2_inst, figog, th = ew(cg2, gb2[p][:, :], t, h2r[t % 2], "2")
                if tc1_inst is not None:
                    # keep next step's tanh(c1) ahead of this step's big L2
                    # sigmoid in the ACT FIFO (it is on the h1 recurrence cycle)
                    add_dep_helper(s2_inst.ins, tc1_inst.ins,
                                   reason="h1-cycle tanh_c before L2 sigmoid")
                if t == T_ - 1:
                    nc.vector.tensor_mul(out_sb, figog[:, 256:384], th)
                    nc.sync.dma_start(out=y_out[:], in_=out_sb)

            # v1-proven software pipeline: L1 of step tau+1 is emitted before
            # L2 of step tau so the PE work between h1(tau) and L1rec(tau+1)
            # is minimal.
            emit_l1(0)
            for tau in range(T_):
                tc1 = emit_l1(tau + 1) if tau + 1 < T_ else None
                emit_l2(tau, tc1)

    nc.compile()
    return nc


_NC_CACHE = {}


def _get_nc(t_steps, with_b1, with_b2):
    key = (t_steps, with_b1, with_b2)
    if key not in _NC_CACHE:
        _NC_CACHE[key] = _build(t_steps, with_b1, with_b2)
    return _NC_CACHE[key]


def _pack_w(W, kchunks):
    """W [128*kchunks, 1024] -> [128, kchunks*8*128] fp16, PERM chunk order,
    with the g-gate chunk columns scaled x2 (tanh-via-sigmoid)."""
    out = np.empty((128, kchunks, 8, 128), dtype=np.float32)
    for k in range(kchunks):
        for j in range(8):
            m = PERM[j]
            w = W[128 * k : 128 * (k + 1), 128 * m : 128 * (m + 1)]
            if j in G_POS:
                w = w * 2.0
            out[:, k, j, :] = w
    return np.ascontiguousarray(out.reshape(128, kchunks * 8 * 128).astype(ml_dtypes.bfloat16))


def _pack_bias(b):
    """b [1024] -> [8, 128] lhsT rows in PERM order (g rows x2)."""
    bb = np.zeros((8, 128), dtype=np.float32)
    for j in range(8):
        bb[j, :] = b[128 * PERM[j] : 128 * (PERM[j] + 1)]
        if j in G_POS:
            bb[j, :] *= 2.0
    return bb.astype(ml_dtypes.bfloat16)


def _make_ind():
    ind = np.zeros((8, 512), dtype=_F16)
    for j in range(8):
        ind[j, 64 * j : 64 * (j + 1)] = 1
    return ind


def _pack_x_core(xc, t_steps):
    """xc [BL, T, D] f32 -> [nblk, 128, XBLK, BL] fp16 (partition = d)."""
    nblk = (t_steps + XBLK - 1) // XBLK
    xt = xc.transpose(1, 2, 0)  # [T, D, BL]
    xt = xt.reshape(nblk, XBLK, D, BL).transpose(0, 2, 1, 3)  # [nblk, D, XBLK, BL]
    return np.ascontiguousarray(xt.astype(_F16))


TRACE = False  # set by test harness to capture a HW profile
LAST_EXEC_NS = None


def kernel(x, W1, b1, W2, b2, Wout, bout):
    global LAST_EXEC_NS
    from concourse.bass_utils import run_bass_kernel_spmd

    x = np.asarray(x)
    W1 = np.asarray(W1, dtype=np.float32)
    b1 = np.asarray(b1, dtype=np.float32)
    W2 = np.asarray(W2, dtype=np.float32)
    b2 = np.asarray(b2, dtype=np.float32)
    Wout = np.asarray(Wout, dtype=np.float32)
    bout = np.asarray(bout, dtype=np.float32)
    t_steps = x.shape[1]

    with_b1 = bool(np.any(b1))
    with_b2 = bool(np.any(b2))
    nc = _get_nc(t_steps, with_b1, with_b2)

    base = {"w1": _pack_w(W1, 3), "w2": _pack_w(W2, 4)}
    if with_b1:
        base["b1"] = _pack_bias(b1)
    if with_b2:
        base["b2"] = _pack_bias(b2)
    if with_b1 or with_b2:
        base["ind"] = _make_ind()

    in_maps = []
    for i in range(NCORES):
        m = dict(base)
        m["x"] = _pack_x_core(x[i * BL : (i + 1) * BL].astype(np.float32), t_steps)
        in_maps.append(m)

    res = run_bass_kernel_spmd(nc, in_maps, list(range(NCORES)), trace=TRACE)
    LAST_EXEC_NS = res.exec_time_ns

    h2 = np.concatenate(
        [
            res.results[i]["y"].reshape(128, 2, 64).transpose(2, 1, 0).reshape(64, 256)
            for i in range(NCORES)
        ],
        axis=0,
    )
    return (h2.astype(np.float32) @ Wout + bout).astype(np.float32)
